# revision 25
# baseline (speedup 1.0000x reference)
"""ViT multi-head attention (B=64, S=197, D=768, H=12) on 8 TRN2 NeuronCores.

Strategy: data-parallel over batch (8 batch elements per core), processed in
batch-PAIRS so most engine ops cover N=2*197=394 columns. Per pair, a fused
Bass/Tile pipeline computes the QKV projections, per-head
softmax(QK^T/8)V in a transposed score layout (no attention transposes),
the CLS-to-patch attention map, and the output projection.

Layouts (bp = batch pair, b in {0,1} within pair):
  inputs (host pre-transposed+paired): qT/kT/vT [4, D=768, 2S=394]
  weights (host pre-transposed): wqT/wkT/wvT/woT [D, D] ([in_dim, out_dim])
  QT, KT  [e, (b s)] e on partitions, 6 tiles [128, 394]; head h = tile h//2,
          partitions (h%2)*64..+64 (row-packs the PE for score matmuls)
  V       [s, e] token-major per b (2 partition chunks: 128 + 69)
  scoresT [k, (b q)] per head: one [cl, 394] PSUM bank, 2 matmuls (b0|b1)
  exp     one ACT pass per chunk with the 1/8 scale folded in
  denom   gpsimd partition_all_reduce over k chunks (broadcast col-sums),
          summed + reciprocal_approx on DVE
  xT      [dh, (b q)] head pairs col-packed into one [128, 512] PSUM bank;
          normalized by DVE mul (PSUM in0 + base-0 SBUF reciprocal in1)
  out     [s, e] per b via Wo matmul, PSUM -> SBUF -> DRAM
  cls     [12, 196] per b: exp column q=0 gathered, PE-transposed, re-summed
          along the free dim for its own denominators, normalized
"""

import os
import numpy as np
from contextlib import ExitStack

import concourse.bass as bass
import concourse.bacc as bacc
import concourse.mybir as mybir
import concourse.bass_isa as bass_isa
import concourse.tile as tile
from concourse.masks import make_identity
from concourse.bass_utils import run_bass_kernel_spmd

F32 = mybir.dt.float32
AF = mybir.ActivationFunctionType

B = 64
H = 12
D = 768
DH = 64
S = 197
S2 = 2 * S
P = 14
NCORES = 8
BL = B // NCORES            # batch elements per core
NBP = BL // 2               # batch pairs per core
ND = 6                      # number of 128-wide d/e chunks
S_CHUNKS = [(0, 128), (128, 69)]
N_CHUNKS = [(0, 512), (512, 256)]
SCALE = 0.125               # 1/sqrt(DH)


def _install_profile_hook():
    """Provide the antenv.axon_hooks module missing from this image so
    run_bass_kernel_spmd(trace=True) can reach NTFF profiling."""
    import sys, types
    if "antenv.axon_hooks" in sys.modules:
        return
    mod = types.ModuleType("antenv.axon_hooks")
    mod._hook = None
    mod.set_axon_ntff_profile_hook = lambda h: setattr(mod, "_hook", h)
    mod.get_axon_ntff_profile_hook = lambda: mod._hook
    sys.modules["antenv.axon_hooks"] = mod
    try:
        import antenv
        antenv.axon_hooks = mod
        from trn_agent_boot.trn_boot import _ntff_profile_via_ctypes
        mod.set_axon_ntff_profile_hook(
            _ntff_profile_via_ctypes('/opt/axon/libaxon_pjrt.so'))
    except Exception:
        pass


def build_kernel():
    nc = bacc.Bacc("TRN2", target_bir_lowering=False, debug=False,
                   enable_asserts=True, num_devices=1)

    qT = nc.dram_tensor("qT", [NBP, D, S2], F32, kind="ExternalInput").ap()
    kT = nc.dram_tensor("kT", [NBP, D, S2], F32, kind="ExternalInput").ap()
    vT = nc.dram_tensor("vT", [NBP, D, S2], F32, kind="ExternalInput").ap()
    wqT = nc.dram_tensor("wqT", [D, D], F32, kind="ExternalInput").ap()
    wkT = nc.dram_tensor("wkT", [D, D], F32, kind="ExternalInput").ap()
    wvT = nc.dram_tensor("wvT", [D, D], F32, kind="ExternalInput").ap()
    woT = nc.dram_tensor("woT", [D, D], F32, kind="ExternalInput").ap()
    out = nc.dram_tensor("out", [BL, S, D], F32, kind="ExternalOutput").ap()
    cls = nc.dram_tensor("cls", [BL, H, S - 1], F32, kind="ExternalOutput").ap()

    with ExitStack() as ctx:
        tc = ctx.enter_context(tile.TileContext(nc))

        consts = ctx.enter_context(tc.tile_pool(name="consts", bufs=1))
        wpool = ctx.enter_context(tc.tile_pool(name="w", bufs=1))
        inp = ctx.enter_context(tc.tile_pool(name="inp", bufs=1))
        qksb = ctx.enter_context(tc.tile_pool(name="qksb", bufs=2))
        vsb = ctx.enter_context(tc.tile_pool(name="vsb", bufs=1))
        expp = ctx.enter_context(tc.tile_pool(name="expp", bufs=2))
        xtp = ctx.enter_context(tc.tile_pool(name="xtp", bufs=2))
        smal = ctx.enter_context(tc.tile_pool(name="smal", bufs=2))
        clsp = ctx.enter_context(tc.tile_pool(name="clsp", bufs=2))

        pp = ctx.enter_context(tc.tile_pool(name="pp", bufs=2, space="PSUM"))
        ps = ctx.enter_context(tc.tile_pool(name="ps", bufs=3, space="PSUM"))
        px = ctx.enter_context(tc.tile_pool(name="px", bufs=2, space="PSUM"))

        # constants
        ident = consts.tile([128, 128], F32)
        make_identity(nc, ident)
        ones = consts.tile([128, 1], F32)
        nc.gpsimd.memset(ones[:, :], 1.0)
        ones_row = consts.tile([1, 64], F32)
        nc.gpsimd.memset(ones_row[:, :], 1.0)

        # weights: 6 tiles of [128, 768] each
        w_sb = {}
        for name, t in (("q", wqT), ("k", wkT), ("v", wvT), ("o", woT)):
            tiles = []
            for kd in range(ND):
                wt = wpool.tile([128, D], F32, tag=f"w{name}{kd}")
                nc.gpsimd.dma_start(wt[:, :], t[kd * 128:(kd + 1) * 128, :])
                tiles.append(wt)
            w_sb[name] = tiles

        st = {}

        def emit_in_qk(bp):
            xq, xk, xv = [], [], []
            for nm, dram, lst in (("q", qT, xq), ("k", kT, xk), ("v", vT, xv)):
                for kd in range(ND):
                    xt_ = inp.tile([128, S2], F32, tag=f"x{nm}{kd}", name=f"x{nm}{kd}")
                    nc.sync.dma_start(xt_[:, :], dram[bp, kd * 128:(kd + 1) * 128, :])
                    lst.append(xt_)
            QT, KT = [], []
            for nm, x_in, lst in (("q", xq, QT), ("k", xk, KT)):
                for m in range(ND):
                    psq = pp.tile([128, 512], F32, tag="pa", name="psq")
                    for kd in range(ND):
                        nc.tensor.matmul(
                            psq[:, :S2],
                            lhsT=w_sb[nm][kd][:, m * 128:(m + 1) * 128],
                            rhs=x_in[kd][:, :],
                            start=(kd == 0), stop=(kd == ND - 1))
                    sb = qksb.tile([128, S2], F32, tag=f"{nm}T{m}", name=f"{nm}T{m}")
                    nc.vector.tensor_copy(sb[:, :], psq[:, :S2])
                    lst.append(sb)
            st[bp] = {"xv": xv, "QT": QT, "KT": KT}

        def emit_v(bp):
            xv = st[bp]["xv"]
            V = []
            for b in range(2):
                vtiles = []
                for c, (s0, sl) in enumerate(S_CHUNKS):
                    sbv = vsb.tile([128, D], F32, tag=f"v{b}{c}", name=f"v{b}{c}")
                    for n0, nl in N_CHUNKS:
                        psv = pp.tile([128, 512], F32, tag="pa", name="psv")
                        for kd in range(ND):
                            nc.tensor.matmul(
                                psv[:sl, :nl],
                                lhsT=xv[kd][:, b * S + s0:b * S + s0 + sl],
                                rhs=w_sb["v"][kd][:, n0:n0 + nl],
                                start=(kd == 0), stop=(kd == ND - 1))
                        nc.scalar.copy(sbv[:sl, n0:n0 + nl], psv[:sl, :nl])
                    vtiles.append(sbv)
                V.append(vtiles)
            st[bp]["V"] = V

        def emit_attn(bp):
            QT, KT, V = st[bp]["QT"], st[bp]["KT"], st[bp]["V"]
            clsg = [[clsp.tile([128, H], F32, tag=f"cg{b}{c}", name=f"clsg{b}{c}")
                     for c in range(2)] for b in range(2)]
            xT = []
            for hp in range(H // 2):
                h0 = 2 * hp
                qt_t, kt_t = QT[hp], KT[hp]

                exs = [[None, None], [None, None]]
                sums = [[None, None], [None, None]]
                for c, (c0, cl) in enumerate(S_CHUNKS):
                    scs = [ps.tile([128, 512], F32, tag="s", name=f"sc{i}")
                           for i in range(2)]
                    for b in range(2):
                        for hi in range(2):
                            po = hi * 64
                            nc.tensor.matmul(
                                scs[hi][:cl, b * S:b * S + S],
                                lhsT=kt_t[po:po + 64, b * S + c0:b * S + c0 + cl],
                                rhs=qt_t[po:po + 64, b * S:b * S + S],
                                start=True, stop=True)
                    for hi in range(2):
                        ex = expp.tile([128, S2], F32, tag=f"e{c}{hi}",
                                       name=f"ex{c}{hi}")
                        nc.scalar.activation(ex[:cl, :], scs[hi][:cl, :S2],
                                             AF.Exp, scale=SCALE)
                        for b in range(2):
                            nc.vector.tensor_copy(
                                clsg[b][c][:cl, h0 + hi:h0 + hi + 1],
                                ex[:cl, b * S:b * S + 1])
                        sm = smal.tile([128, S2], F32, tag=f"sm{c}{hi}",
                                       name=f"sm{c}{hi}", bufs=1)
                        nc.gpsimd.partition_all_reduce(
                            sm[:cl, :], ex[:cl, :], channels=cl,
                            reduce_op=bass_isa.ReduceOp.add)
                        exs[hi][c] = ex
                        sums[hi][c] = sm

                recs = []
                for hi in range(2):
                    tot = smal.tile([64, S2], F32, tag=f"tot{hi}", bufs=1,
                                    name=f"tot{hi}")
                    nc.vector.tensor_add(tot[:, :], sums[hi][0][0:64, :],
                                         sums[hi][1][0:64, :])
                    bcs = smal.tile([64, S2], F32, tag=f"bcs{hi}", bufs=2,
                                    name=f"bcs{hi}")
                    scr = smal.tile([64, S2], F32, tag="scr", bufs=1, name="scr")
                    nc.vector.reciprocal_approx_accurate(bcs[:, :], tot[:, :],
                                                         scr[:, :])
                    recs.append(bcs)

                xps = px.tile([128, 512], F32, tag="x", name="xps")
                for hi in range(2):
                    po = hi * 64
                    for b in range(2):
                        for c, (c0, cl) in enumerate(S_CHUNKS):
                            nc.tensor.matmul(
                                xps[po:po + 64, b * S:b * S + S],
                                lhsT=V[b][c][:cl, (h0 + hi) * 64:(h0 + hi) * 64 + 64],
                                rhs=exs[hi][c][:cl, b * S:b * S + S],
                                start=(c == 0), stop=(c == 1))

                xsb = xtp.tile([128, S2], F32, tag=f"xt{hp}", name=f"xt{hp}")
                xT.append(xsb)
                for hi in range(2):
                    po = hi * 64
                    nc.vector.tensor_mul(xsb[po:po + 64, :],
                                         xps[po:po + 64, :S2], recs[hi][:, :])
            st[bp]["xT"] = xT
            st[bp]["clsg"] = clsg

        def emit_cls(bp):
            clsg = st[bp]["clsg"]
            for b in range(2):
                pcls = ps.tile([128, 512], F32, tag="pc", bufs=1, name="pcls")
                nc.tensor.transpose(pcls[:H, 0:128], clsg[b][0][:, :], ident[:, :])
                nc.tensor.transpose(pcls[:H, 128:197], clsg[b][1][0:69, :],
                                    ident[0:69, 0:69])
                clsden = smal.tile([H, 1], F32, tag="cd", name="clsden")
                nc.vector.reduce_sum(clsden[:, :], pcls[:H, 0:S],
                                     axis=mybir.AxisListType.X)
                clsrec = smal.tile([H, 1], F32, tag="cr", name="clsrec")
                clsscr = smal.tile([H, 1], F32, tag="cscr", name="clsscr")
                nc.vector.reciprocal_approx_accurate(clsrec[:, :], clsden[:, :],
                                                     clsscr[:, :])
                cls_sb = clsp.tile([H, S - 1], F32, tag="cs", name="cls_sb")
                nc.vector.tensor_scalar_mul(cls_sb[:, :], pcls[:H, 1:S], clsrec[:, :])
                nc.sync.dma_start(cls[2 * bp + b], cls_sb[:, :])

        def emit_wo(bp):
            xT = st[bp]["xT"]
            for b in range(2):
                for s0, sl in S_CHUNKS:
                    for n0, nl in N_CHUNKS:
                        pso = pp.tile([128, 512], F32, tag="pa", name="pso")
                        for kd in range(ND):
                            nc.tensor.matmul(
                                pso[:sl, :nl],
                                lhsT=xT[kd][:, b * S + s0:b * S + s0 + sl],
                                rhs=w_sb["o"][kd][:, n0:n0 + nl],
                                start=(kd == 0), stop=(kd == ND - 1))
                        osb = clsp.tile([128, 512], F32, tag="osb", name="osb")
                        nc.scalar.copy(osb[:sl, :nl], pso[:sl, :nl])
                        nc.sync.dma_start(out[2 * bp + b, s0:s0 + sl, n0:n0 + nl],
                                          osb[:sl, :nl])

        for bp in range(NBP):
            emit_in_qk(bp)
            if bp > 0:
                emit_attn(bp - 1)
            emit_v(bp)
            if bp > 0:
                emit_wo(bp - 1)
                emit_cls(bp - 1)
        emit_attn(NBP - 1)
        emit_wo(NBP - 1)
        emit_cls(NBP - 1)

    nc.compile()
    return nc


_NC = None


def _get_nc():
    global _NC
    if _NC is None:
        _NC = build_kernel()
    return _NC


def kernel(q, k, v, Wq, Wk, Wv, Wo):
    q = np.asarray(q, np.float32)
    k = np.asarray(k, np.float32)
    v = np.asarray(v, np.float32)

    def pair(x):
        # [B, S, D] -> [B/2, D, 2S] with x_p[i, d, b*S+s] = x[2i+b, s, d]
        xt = x.transpose(0, 2, 1).reshape(B // 2, 2, D, S)
        return np.ascontiguousarray(xt.transpose(0, 2, 1, 3).reshape(B // 2, D, S2))

    qT = pair(q)
    kTh = pair(k)
    vT = pair(v)
    wqT = np.ascontiguousarray(np.asarray(Wq, np.float32).T)
    wkT = np.ascontiguousarray(np.asarray(Wk, np.float32).T)
    wvT = np.ascontiguousarray(np.asarray(Wv, np.float32).T)
    woT = np.ascontiguousarray(np.asarray(Wo, np.float32).T)

    in_maps = []
    for c in range(NCORES):
        sl = slice(c * NBP, (c + 1) * NBP)
        in_maps.append({
            "qT": qT[sl], "kT": kTh[sl], "vT": vT[sl],
            "wqT": wqT, "wkT": wkT, "wvT": wvT, "woT": woT,
        })

    nc = _get_nc()
    profile = os.environ.get("MHA_PROFILE", "") == "1"
    if profile:
        _install_profile_hook()
    res = run_bass_kernel_spmd(nc, in_maps, list(range(NCORES)),
                               trace=profile, trace_cores=[0] if profile else None)
    if profile and res.exec_time_ns is not None:
        print(f"HW exec time: {res.exec_time_ns} ns")

    output = np.concatenate([res.results[c]["out"] for c in range(NCORES)], axis=0)
    cls_map = np.concatenate([res.results[c]["cls"] for c in range(NCORES)], axis=0)
    return output, cls_map.reshape(B, H, P, P)


# revision 26
# speedup vs baseline: 1.0610x; 1.0610x over previous
"""ViT multi-head attention (B=64, S=197, D=768, H=12) on 8 TRN2 NeuronCores.

Strategy: data-parallel over batch (8 batch elements per core), processed in
batch-PAIRS so most engine ops cover N=2*197=394 columns. Per pair, a fused
Bass/Tile pipeline computes the QKV projections, per-head
softmax(QK^T/8)V in a transposed score layout (no attention transposes),
the CLS-to-patch attention map, and the output projection.

Layouts (bp = batch pair, b in {0,1} within pair):
  inputs (host pre-transposed+paired): qT/kT/vT [4, D=768, 2S=394]
  weights (host pre-transposed): wqT/wkT/wvT/woT [D, D] ([in_dim, out_dim])
  QT, KT  [e, (b s)] e on partitions, 6 tiles [128, 394]; head h = tile h//2,
          partitions (h%2)*64..+64 (row-packs the PE for score matmuls)
  V       [s, e] token-major per b (2 partition chunks: 128 + 69)
  scoresT [k, (b q)] per head: one [cl, 394] PSUM bank, 2 matmuls (b0|b1)
  exp     one ACT pass per chunk with the 1/8 scale folded in
  denom   gpsimd partition_all_reduce over k chunks (broadcast col-sums),
          summed + reciprocal_approx on DVE
  xT      [dh, (b q)] head pairs col-packed into one [128, 512] PSUM bank;
          normalized by DVE mul (PSUM in0 + base-0 SBUF reciprocal in1)
  out     [s, e] per b via Wo matmul, PSUM -> SBUF -> DRAM
  cls     [12, 196] per b: exp column q=0 gathered, PE-transposed, re-summed
          along the free dim for its own denominators, normalized
"""

import os
import numpy as np
from contextlib import ExitStack

import concourse.bass as bass
import concourse.bacc as bacc
import concourse.mybir as mybir
import concourse.bass_isa as bass_isa
import concourse.tile as tile
from concourse.masks import make_identity
from concourse.bass_utils import run_bass_kernel_spmd

F32 = mybir.dt.float32
AF = mybir.ActivationFunctionType

B = 64
H = 12
D = 768
DH = 64
S = 197
S2 = 2 * S
P = 14
NCORES = 8
BL = B // NCORES            # batch elements per core
NBP = BL // 2               # batch pairs per core
ND = 6                      # number of 128-wide d/e chunks
S_CHUNKS = [(0, 128), (128, 69)]
N_CHUNKS = [(0, 512), (512, 256)]
SCALE = 0.125               # 1/sqrt(DH)


def _install_profile_hook():
    """Provide the antenv.axon_hooks module missing from this image so
    run_bass_kernel_spmd(trace=True) can reach NTFF profiling."""
    import sys, types
    if "antenv.axon_hooks" in sys.modules:
        return
    mod = types.ModuleType("antenv.axon_hooks")
    mod._hook = None
    mod.set_axon_ntff_profile_hook = lambda h: setattr(mod, "_hook", h)
    mod.get_axon_ntff_profile_hook = lambda: mod._hook
    sys.modules["antenv.axon_hooks"] = mod
    try:
        import antenv
        antenv.axon_hooks = mod
        from trn_agent_boot.trn_boot import _ntff_profile_via_ctypes
        mod.set_axon_ntff_profile_hook(
            _ntff_profile_via_ctypes('/opt/axon/libaxon_pjrt.so'))
    except Exception:
        pass


def build_kernel():
    nc = bacc.Bacc("TRN2", target_bir_lowering=False, debug=False,
                   enable_asserts=True, num_devices=1)

    qT = nc.dram_tensor("qT", [NBP, D, S2], F32, kind="ExternalInput").ap()
    kT = nc.dram_tensor("kT", [NBP, D, S2], F32, kind="ExternalInput").ap()
    vT = nc.dram_tensor("vT", [NBP, D, S2], F32, kind="ExternalInput").ap()
    wqT = nc.dram_tensor("wqT", [D, D], F32, kind="ExternalInput").ap()
    wkT = nc.dram_tensor("wkT", [D, D], F32, kind="ExternalInput").ap()
    wvT = nc.dram_tensor("wvT", [D, D], F32, kind="ExternalInput").ap()
    woT = nc.dram_tensor("woT", [D, D], F32, kind="ExternalInput").ap()
    out = nc.dram_tensor("out", [BL, S, D], F32, kind="ExternalOutput").ap()
    cls = nc.dram_tensor("cls", [BL, H, S - 1], F32, kind="ExternalOutput").ap()

    with ExitStack() as ctx:
        tc = ctx.enter_context(tile.TileContext(nc))

        consts = ctx.enter_context(tc.tile_pool(name="consts", bufs=1))
        wpool = ctx.enter_context(tc.tile_pool(name="w", bufs=1))
        inp = ctx.enter_context(tc.tile_pool(name="inp", bufs=1))
        qksb = ctx.enter_context(tc.tile_pool(name="qksb", bufs=2))
        vsb = ctx.enter_context(tc.tile_pool(name="vsb", bufs=1))
        expp = ctx.enter_context(tc.tile_pool(name="expp", bufs=2))
        xtp = ctx.enter_context(tc.tile_pool(name="xtp", bufs=2))
        smal = ctx.enter_context(tc.tile_pool(name="smal", bufs=2))
        clsp = ctx.enter_context(tc.tile_pool(name="clsp", bufs=2))

        pp = ctx.enter_context(tc.tile_pool(name="pp", bufs=2, space="PSUM"))
        ps = ctx.enter_context(tc.tile_pool(name="ps", bufs=2, space="PSUM"))
        px = ctx.enter_context(tc.tile_pool(name="px", bufs=3, space="PSUM"))

        # constants
        ident = consts.tile([128, 128], F32)
        make_identity(nc, ident)
        ones = consts.tile([128, 1], F32)
        nc.gpsimd.memset(ones[:, :], 1.0)
        ones_row = consts.tile([1, 64], F32)
        nc.gpsimd.memset(ones_row[:, :], 1.0)

        # weights: 6 tiles of [128, 768] each
        w_sb = {}
        for name, t in (("q", wqT), ("k", wkT), ("v", wvT), ("o", woT)):
            tiles = []
            for kd in range(ND):
                wt = wpool.tile([128, D], F32, tag=f"w{name}{kd}")
                nc.gpsimd.dma_start(wt[:, :], t[kd * 128:(kd + 1) * 128, :])
                tiles.append(wt)
            w_sb[name] = tiles

        st = {}

        def emit_in_qk(bp):
            xq, xk, xv = [], [], []
            for nm, dram, lst in (("q", qT, xq), ("k", kT, xk), ("v", vT, xv)):
                for kd in range(ND):
                    xt_ = inp.tile([128, S2], F32, tag=f"x{nm}{kd}", name=f"x{nm}{kd}")
                    nc.sync.dma_start(xt_[:, :], dram[bp, kd * 128:(kd + 1) * 128, :])
                    lst.append(xt_)
            QT, KT = [], []
            for nm, x_in, lst in (("q", xq, QT), ("k", xk, KT)):
                for m in range(ND):
                    psq = pp.tile([128, 512], F32, tag="pa", name="psq")
                    for kd in range(ND):
                        nc.tensor.matmul(
                            psq[:, :S2],
                            lhsT=w_sb[nm][kd][:, m * 128:(m + 1) * 128],
                            rhs=x_in[kd][:, :],
                            start=(kd == 0), stop=(kd == ND - 1))
                    sb = qksb.tile([128, S2], F32, tag=f"{nm}T{m}", name=f"{nm}T{m}")
                    nc.vector.tensor_copy(sb[:, :], psq[:, :S2])
                    lst.append(sb)
            st[bp] = {"xv": xv, "QT": QT, "KT": KT}

        def emit_v(bp):
            xv = st[bp]["xv"]
            V = []
            for b in range(2):
                vtiles = []
                for c, (s0, sl) in enumerate(S_CHUNKS):
                    sbv = vsb.tile([128, D], F32, tag=f"v{b}{c}", name=f"v{b}{c}")
                    for n0, nl in N_CHUNKS:
                        psv = pp.tile([128, 512], F32, tag="pa", name="psv")
                        for kd in range(ND):
                            nc.tensor.matmul(
                                psv[:sl, :nl],
                                lhsT=xv[kd][:, b * S + s0:b * S + s0 + sl],
                                rhs=w_sb["v"][kd][:, n0:n0 + nl],
                                start=(kd == 0), stop=(kd == ND - 1))
                        nc.scalar.copy(sbv[:sl, n0:n0 + nl], psv[:sl, :nl])
                    vtiles.append(sbv)
                V.append(vtiles)
            st[bp]["V"] = V

        def emit_attn(bp):
            QT, KT, V = st[bp]["QT"], st[bp]["KT"], st[bp]["V"]
            clsg = [[clsp.tile([128, H], F32, tag=f"cg{b}{c}", name=f"clsg{b}{c}")
                     for c in range(2)] for b in range(2)]
            xT = []
            for hp in range(H // 2):
                h0 = 2 * hp
                qt_t, kt_t = QT[hp], KT[hp]

                exs = [[None, None], [None, None]]
                sums = [[None, None], [None, None]]
                for c, (c0, cl) in enumerate(S_CHUNKS):
                    scs = [ps.tile([128, 512], F32, tag="s", name=f"sc{i}")
                           for i in range(2)]
                    for b in range(2):
                        for hi in range(2):
                            po = hi * 64
                            nc.tensor.matmul(
                                scs[hi][:cl, b * S:b * S + S],
                                lhsT=kt_t[po:po + 64, b * S + c0:b * S + c0 + cl],
                                rhs=qt_t[po:po + 64, b * S:b * S + S],
                                start=True, stop=True)
                    for hi in range(2):
                        ex = expp.tile([128, S2], F32, tag=f"e{c}{hi}",
                                       name=f"ex{c}{hi}")
                        nc.scalar.activation(ex[:cl, :], scs[hi][:cl, :S2],
                                             AF.Exp, scale=SCALE)
                        for b in range(2):
                            nc.vector.tensor_copy(
                                clsg[b][c][:cl, h0 + hi:h0 + hi + 1],
                                ex[:cl, b * S:b * S + 1])
                        sm = smal.tile([128, S2], F32, tag=f"sm{c}{hi}",
                                       name=f"sm{c}{hi}", bufs=1)
                        nc.gpsimd.partition_all_reduce(
                            sm[:cl, :], ex[:cl, :], channels=cl,
                            reduce_op=bass_isa.ReduceOp.add)
                        exs[hi][c] = ex
                        sums[hi][c] = sm

                recs = []
                for hi in range(2):
                    tot = smal.tile([64, S2], F32, tag=f"tot{hi}", bufs=1,
                                    name=f"tot{hi}")
                    nc.vector.tensor_add(tot[:, :], sums[hi][0][0:64, :],
                                         sums[hi][1][0:64, :])
                    bcs = smal.tile([64, S2], F32, tag=f"bcs{hi}", bufs=2,
                                    name=f"bcs{hi}")
                    scr = smal.tile([64, S2], F32, tag="scr", bufs=1, name="scr")
                    nc.vector.reciprocal_approx_accurate(bcs[:, :], tot[:, :],
                                                         scr[:, :])
                    recs.append(bcs)

                xps = px.tile([128, 512], F32, tag="x", name="xps")
                for hi in range(2):
                    po = hi * 64
                    for b in range(2):
                        for c, (c0, cl) in enumerate(S_CHUNKS):
                            nc.tensor.matmul(
                                xps[po:po + 64, b * S:b * S + S],
                                lhsT=V[b][c][:cl, (h0 + hi) * 64:(h0 + hi) * 64 + 64],
                                rhs=exs[hi][c][:cl, b * S:b * S + S],
                                start=(c == 0), stop=(c == 1))

                xsb = xtp.tile([128, S2], F32, tag=f"xt{hp}", name=f"xt{hp}")
                xT.append(xsb)
                for hi in range(2):
                    po = hi * 64
                    nc.vector.tensor_mul(xsb[po:po + 64, :],
                                         xps[po:po + 64, :S2], recs[hi][:, :])
            st[bp]["xT"] = xT
            st[bp]["clsg"] = clsg

        def emit_cls(bp):
            clsg = st[bp]["clsg"]
            for b in range(2):
                pcls = ps.tile([128, 512], F32, tag="pc", bufs=1, name="pcls")
                nc.tensor.transpose(pcls[:H, 0:128], clsg[b][0][:, :], ident[:, :])
                nc.tensor.transpose(pcls[:H, 128:197], clsg[b][1][0:69, :],
                                    ident[0:69, 0:69])
                clsden = smal.tile([H, 1], F32, tag="cd", name="clsden")
                nc.vector.reduce_sum(clsden[:, :], pcls[:H, 0:S],
                                     axis=mybir.AxisListType.X)
                clsrec = smal.tile([H, 1], F32, tag="cr", name="clsrec")
                clsscr = smal.tile([H, 1], F32, tag="cscr", name="clsscr")
                nc.vector.reciprocal_approx_accurate(clsrec[:, :], clsden[:, :],
                                                     clsscr[:, :])
                cls_sb = clsp.tile([H, S - 1], F32, tag="cs", name="cls_sb")
                nc.vector.tensor_scalar_mul(cls_sb[:, :], pcls[:H, 1:S], clsrec[:, :])
                nc.sync.dma_start(cls[2 * bp + b], cls_sb[:, :])

        def emit_wo(bp):
            xT = st[bp]["xT"]
            for b in range(2):
                for s0, sl in S_CHUNKS:
                    for n0, nl in N_CHUNKS:
                        pso = pp.tile([128, 512], F32, tag="pa", name="pso")
                        for kd in range(ND):
                            nc.tensor.matmul(
                                pso[:sl, :nl],
                                lhsT=xT[kd][:, b * S + s0:b * S + s0 + sl],
                                rhs=w_sb["o"][kd][:, n0:n0 + nl],
                                start=(kd == 0), stop=(kd == ND - 1))
                        osb = clsp.tile([128, 512], F32, tag="osb", name="osb")
                        nc.scalar.copy(osb[:sl, :nl], pso[:sl, :nl])
                        nc.sync.dma_start(out[2 * bp + b, s0:s0 + sl, n0:n0 + nl],
                                          osb[:sl, :nl])

        for bp in range(NBP):
            emit_in_qk(bp)
            if bp > 0:
                emit_attn(bp - 1)
            emit_v(bp)
            if bp > 0:
                emit_wo(bp - 1)
                emit_cls(bp - 1)
        emit_attn(NBP - 1)
        emit_wo(NBP - 1)
        emit_cls(NBP - 1)

    nc.compile()
    return nc


_NC = None


def _get_nc():
    global _NC
    if _NC is None:
        _NC = build_kernel()
    return _NC


def kernel(q, k, v, Wq, Wk, Wv, Wo):
    q = np.asarray(q, np.float32)
    k = np.asarray(k, np.float32)
    v = np.asarray(v, np.float32)

    def pair(x):
        # [B, S, D] -> [B/2, D, 2S] with x_p[i, d, b*S+s] = x[2i+b, s, d]
        xt = x.transpose(0, 2, 1).reshape(B // 2, 2, D, S)
        return np.ascontiguousarray(xt.transpose(0, 2, 1, 3).reshape(B // 2, D, S2))

    qT = pair(q)
    kTh = pair(k)
    vT = pair(v)
    wqT = np.ascontiguousarray(np.asarray(Wq, np.float32).T)
    wkT = np.ascontiguousarray(np.asarray(Wk, np.float32).T)
    wvT = np.ascontiguousarray(np.asarray(Wv, np.float32).T)
    woT = np.ascontiguousarray(np.asarray(Wo, np.float32).T)

    in_maps = []
    for c in range(NCORES):
        sl = slice(c * NBP, (c + 1) * NBP)
        in_maps.append({
            "qT": qT[sl], "kT": kTh[sl], "vT": vT[sl],
            "wqT": wqT, "wkT": wkT, "wvT": wvT, "woT": woT,
        })

    nc = _get_nc()
    profile = os.environ.get("MHA_PROFILE", "") == "1"
    if profile:
        _install_profile_hook()
    res = run_bass_kernel_spmd(nc, in_maps, list(range(NCORES)),
                               trace=profile, trace_cores=[0] if profile else None)
    if profile and res.exec_time_ns is not None:
        print(f"HW exec time: {res.exec_time_ns} ns")

    output = np.concatenate([res.results[c]["out"] for c in range(NCORES)], axis=0)
    cls_map = np.concatenate([res.results[c]["cls"] for c in range(NCORES)], axis=0)
    return output, cls_map.reshape(B, H, P, P)
